# revision 17
# baseline (speedup 1.0000x reference)
"""Trainium2 Bass kernel for nn_KANLayer (piecewise-constant KAN forward).

Math: reference computes out[t,i] = sum_j f[i,j,m(x_tj)] where m = segment(x)
in 0..8 and f[i,j,m] = c_m + c_{m+1} + c_{m+2} (9-valued selection). The whole
contraction runs in fp8-e4m3 DoubleRow (K=256 per 213ns N=512 matmul):

    out[t,i] = base_i + sum_a co_a[i,j]*phi_a(m_tj)   eigen-planes
             + sum_{m!=4} R[i,j,m] * onehot_m(t,j)    8 planes, 16 units

phi1/phi2 are the top-2 eigenvectors of the (m!=4) covariance of
D = f - f(4), snapped to fp8-exact values (they capture ~83%% of residual
variance vs ~64%% for a lin/quad pair). phi1 covers both j-halves (2 units);
phi2 only the first jc-pair (1 unit, 19 units total) -- the second half's
loadings refit with phi1 alone; host-sim rel err 1.72e-2 vs the 2e-2 gate.
Tables are quantized scale-free (fp8 is floating point, per-row scaling buys
nothing), with the eigen loadings quantized first so their error is absorbed
by the later-quantized one-hot residual R; residual at m=4 is exactly zero
(base anchored at f(4)), so the m=4 plane is dropped. The m7/m8 one-hot
planes are device-built (DVE bf16 is_equal on the phi2 plane -- its snapped
values are kept distinct -- then ACT copy-convert to fp8); everything else
ships as raw e4m3 bytes from host.

Schedule per core: 19 units x 4 out-blocks x 4 token-groups = 304 DR matmuls
(~65us PE). Every (ob, tg) accumulator splits into gen-A (12 early-DMA
units) and gen-B (7 late units: m6 + phi2 + device-built m7/m8). gen-A
partials spill to SBUF f32 with the output bias pre-added (one DVE op), so
the PE has 16 real work streams on 8 PSUM banks and needs no warmup spin
while input DMA ramps. Input pieces complete FIFO per issue queue and the
16 shared rings wake in fixed waves (~2.6/5.5/8.5us), so planes ship as
524KB jc-pair pieces (4KB/partition lines) split q0->sync / q1->scalar in
unit-stream order with each unit-pair's 262KB table chunk interleaved just
ahead; only the tiny bias vector rides the slow gpsimd queue. gen-B runs
accumulator-outer so final evacuations (one DVE tensor_tensor add each)
stagger instead of bursting after the last matmul; out DMA is batched per
(ob, tg-pair) and the last slice evacuates in two pipelined quarters.
Output leaves as [out_block, 128i, tok] bf16, upcast/transposed on host.
Sharding: data-parallel over tokens, 2048 per core; tables replicated.
"""

from contextlib import ExitStack

import numpy as np

import concourse.bass as bass  # noqa: F401
import concourse.tile as tile
from concourse import bacc, mybir
from concourse.bass_utils import run_bass_kernel_spmd

N_CORES = 8
TOK = 2048          # tokens per core
IN_F = 512
OUT_F = 512
JC = IN_F // 128    # 4 j-chunks of 128
NPASS = OUT_F // 128  # 4 out-blocks
NTG = 4             # token groups (N=512 matmuls) per out-block
TGW = TOK // NTG
NU = 19             # DR units: phi1 x2 + phi2 x1 + 16 one-hot (m!=4)
UA = 12             # gen-A units: phi1, oh m0..m3, m5
NSHIP = 8           # shipped planes
FP8 = mybir.dt.float8e4
BF16 = mybir.dt.bfloat16
F32 = mybir.dt.float32
E4NP = mybir.dt.np(FP8)  # ml_dtypes.float8_e4m3 (TRN: bias 7, max 240)

# plane slots (pl tensor): 0=phi1, 1=oh m0, 2=oh m1, 3=oh m2, 4=phi2,
# 5=oh m3, 6=oh m5, 7=oh m6, 8=oh m7 (device), 9=oh m8 (device).
# 2-plane DMA pieces: [0:2], [2:4], [4:6], [6:8].
SLOT_OF_OH = {0: 1, 1: 2, 2: 3, 3: 5, 5: 6, 6: 7}  # shipped one-hots

# unit -> (plane slot, jc-pair q), PE stream order. gen-A: phi1, m0..m3, m5
# (12). gen-B: m6, phi2 (q0 only), m7, m8 (7).
_UNITS = [(pk, q) for pk in (0, 1, 2, 3, 5, 6) for q in range(2)] \
    + [(7, 0), (7, 1), (4, 0), (8, 0), (8, 1), (9, 0), (9, 1)]
assert len(_UNITS) == NU

_PROGRAM_CACHE = {}


def _build_program():
    nc = bacc.Bacc("TRN2", target_bir_lowering=False, debug=False)

    pl_d = nc.dram_tensor("pl", [128, NSHIP, JC, TOK], FP8,
                          kind="ExternalInput").ap()
    g_d = nc.dram_tensor("g", [128, NU, 2, NPASS, 128], FP8,
                         kind="ExternalInput").ap()
    sb_d = nc.dram_tensor("sb", [128, NPASS], F32, kind="ExternalInput").ap()
    out_d = nc.dram_tensor("out", [NPASS, 128, TOK], BF16,
                           kind="ExternalOutput").ap()

    with tile.TileContext(nc) as tc, ExitStack() as ctx:
        tmp_pool = ctx.enter_context(tc.tile_pool(name="tmp", bufs=2))
        pl_pool = ctx.enter_context(tc.tile_pool(name="pl", bufs=1))
        g_pool = ctx.enter_context(tc.tile_pool(name="g", bufs=1))
        sb_pool = ctx.enter_context(tc.tile_pool(name="sb", bufs=1))
        part_pool = ctx.enter_context(tc.tile_pool(name="part", bufs=1))
        out_pool = ctx.enter_context(tc.tile_pool(name="out", bufs=4))
        psum_pool = ctx.enter_context(tc.tile_pool(name="psum", bufs=8,
                                                   space="PSUM"))

        # --- input DMAs: few big pieces (descriptor-rate-bound frontend).
        # Planes as four 2-plane pieces in unit-stream order on the two
        # hwdge queues; all tables as one piece + sb on the gpsimd queue.
        pl_t = pl_pool.tile([128, NSHIP + 2, JC, TOK], FP8, name="pl")
        g_t = g_pool.tile([128, NU, 2, NPASS, 128], FP8, name="g")
        sb_t = sb_pool.tile([128, NPASS], F32, name="sb")

        # Queue model (measured): pieces complete FIFO per queue; sync rings
        # wake ~2.7/5.7us, scalar ~8.9us, gpsimd late+slow (only sb goes
        # there). Planes ship as jc-pair halves (524KB, 4KB lines) split
        # q0->sync / q1->scalar in unit-stream order; each unit-pair's
        # tables (262KB chunk) ride the opposite queue just ahead.
        def pq(eng, pk, q):
            eng.dma_start(pl_t[:, pk, 2 * q:2 * q + 2],
                          pl_d[:, pk, 2 * q:2 * q + 2])

        def gq(eng, u0, u1):
            eng.dma_start(g_t[:, u0:u1], g_d[:, u0:u1])

        pq(nc.sync, 0, 0)          # phi1 q0
        pq(nc.scalar, 0, 1)        # phi1 q1
        gq(nc.sync, 0, 1)          # unit-0 table (minimal first-MM gate)
        gq(nc.scalar, 1, 2)        # unit-1 table
        gq(nc.scalar, 2, 4)        # m0 tables
        pq(nc.sync, 1, 0)          # m0 q0
        pq(nc.scalar, 1, 1)        # m0 q1
        gq(nc.sync, 4, 6)          # m1 tables
        gq(nc.scalar, 6, 8)        # m2 tables
        pq(nc.sync, 2, 0)          # m1 q0
        pq(nc.scalar, 2, 1)        # m1 q1
        pq(nc.sync, 3, 0)          # m2 q0
        pq(nc.scalar, 3, 1)        # m2 q1
        gq(nc.sync, 8, 10)         # m3 tables
        nc.gpsimd.dma_start(sb_t[:], sb_d[:])
        pq(nc.scalar, 4, 0)        # phi2 q0 (ACT builds + unit 14)
        pq(nc.sync, 5, 0)          # m3 q0
        pq(nc.scalar, 5, 1)        # m3 q1
        gq(nc.sync, 10, 12)        # m5 tables
        pq(nc.scalar, 4, 1)        # phi2 q1 (ACT builds)
        pq(nc.sync, 6, 0)          # m5 q0
        pq(nc.scalar, 6, 1)        # m5 q1
        gq(nc.sync, 12, 14)        # m6 tables
        gq(nc.scalar, 14, 15)      # phi2 table
        pq(nc.sync, 7, 0)          # m6 q0
        pq(nc.scalar, 7, 1)        # m6 q1
        gq(nc.sync, 15, 19)        # m7/m8 tables

        # Device-built planes: one-hot m7/m8 via DVE bf16 is_equal on the
        # phi2 plane (values kept distinct host-side) + ACT copy-convert to
        # fp8, per jc-pair chunk.
        for slot, mval in ((8, 7), (9, 8)):
            for q in range(2):
                tmp = tmp_pool.tile([128, 2, TOK], BF16, name="ohb")
                nc.vector.tensor_scalar(
                    tmp[:], pl_t[:, 4, 2 * q:2 * q + 2],
                    _PROGRAM_CACHE["phi2_cmp"][mval], None,
                    mybir.AluOpType.is_equal,
                )
                nc.scalar.activation(
                    pl_t[:, slot, 2 * q:2 * q + 2],
                    tmp[:],
                    mybir.ActivationFunctionType.Copy,
                )

        def mm(ps, ob, u, tg, start, stop):
            pk, q = _UNITS[u]
            nc.tensor.matmul(
                ps,
                g_t[:, u, :, ob, :],
                pl_t[:, pk, 2 * q:2 * q + 2, tg * TGW:(tg + 1) * TGW],
                start=start,
                stop=stop,
                perf_mode=mybir.MatmulPerfMode.DoubleRow,
            )

        # part_t[:, h, ob, tgi*TGW:...] = gen-A partial + bias for (ob, tg)
        part_t = part_pool.tile([128, 2, NPASS, TGW * 2], F32, name="part")

        # gen-A: unit-outer (DMA arrival order), tg halves h=0 then h=1.
        pss = {}
        for h in (0, 1):
            for u in range(UA):
                for ob in range(NPASS):
                    for tgi in (0, 1):
                        if u == 0:
                            pss[ob, tgi] = psum_pool.tile(
                                [128, TGW], F32, name="ps")
                        mm(pss[ob, tgi][:], ob, u, 2 * h + tgi,
                           start=(u == 0), stop=(u == UA - 1))
            for ob in range(NPASS):
                for tgi in (0, 1):
                    nc.vector.tensor_scalar(
                        part_t[:, h, ob, tgi * TGW:(tgi + 1) * TGW],
                        pss[ob, tgi][:], sb_t[:, ob:ob + 1], None,
                        mybir.AluOpType.add,
                    )

        # gen-B: accumulator-outer so evacs stagger (one DVE add each); out
        # DMA batched per (h, ob) except the last pair, whose second half
        # evacuates in two pipelined quarters for the shortest exposed tail.
        for h in (0, 1):
            for ob in range(NPASS):
                otb = out_pool.tile([128, 2 * TGW], BF16, name="otb")
                last = (h == 1 and ob == NPASS - 1)
                eng = nc.sync if ob % 2 == 0 else nc.scalar
                for tgi in (0, 1):
                    tg = 2 * h + tgi
                    ps = psum_pool.tile([128, TGW], F32, name="ps")
                    for ui, u in enumerate(range(UA, NU)):
                        mm(ps[:], ob, u, tg,
                           start=(ui == 0), stop=(u == NU - 1))
                    nhalf = 2 if (last and tgi == 1) else 1
                    hw = TGW // nhalf
                    for hh in range(nhalf):
                        sl = slice(tgi * TGW + hh * hw,
                                   tgi * TGW + (hh + 1) * hw)
                        nc.vector.tensor_tensor(
                            otb[:, sl], ps[:, hh * hw:(hh + 1) * hw],
                            part_t[:, h, ob, sl],
                            mybir.AluOpType.add,
                        )
                        if last:
                            # alternate queues so the two final pieces'
                            # issue latencies overlap
                            leng = nc.scalar if hh == 0 else nc.sync
                            leng.dma_start(
                                out_d[ob][:, 2 * h * TGW:][:, sl],
                                otb[:, sl])
                if not last:
                    eng.dma_start(
                        out_d[ob][:, 2 * h * TGW:2 * (h + 1) * TGW], otb[:])

    nc.compile()
    return nc


def _get_program(phi2_cmp=None):
    # phi2 compare constants are baked into the program; rebuild if they
    # change (same coeffs -> same program).
    if phi2_cmp is None:
        return _PROGRAM_CACHE["nc"]
    key = ("nc", tuple(sorted(phi2_cmp.items())))
    if _PROGRAM_CACHE.get("key") != key:
        _PROGRAM_CACHE["phi2_cmp"] = phi2_cmp
        _PROGRAM_CACHE["nc"] = _build_program()
        _PROGRAM_CACHE["key"] = key
    return _PROGRAM_CACHE["nc"]


def _plane_dev(arr):
    """[T_all, IN] -> [128, JC, T_all] device layout (j = jc*128 + p)."""
    return np.ascontiguousarray(arr.T.reshape(JC, 128, -1).transpose(1, 0, 2))


def _pack_pair(tab_b):
    """e4m3 [OUT, IN] -> [128p, 2q, 2e, NPASS, 128col] stationary layout."""
    t = tab_b.reshape(NPASS, 128, JC, 128).transpose(3, 2, 0, 1)
    return np.ascontiguousarray(t.reshape(128, 2, 2, NPASS, 128))


def _fp8_grid():
    b = np.arange(256, dtype=np.uint8).view(E4NP).astype(np.float64)
    return np.unique(b[np.isfinite(b)])


def _snap_phi(phi):
    """Snap phi (phi[4]=0 preserved) to fp8-exact values, scaled to ~12."""
    ph = (phi * (12.0 / np.abs(phi).max())).astype(E4NP).astype(np.float64)
    ph[4] = 0.0
    return ph


def kernel(x: np.ndarray, coeffs: np.ndarray) -> np.ndarray:
    assert x.shape == (8, 2048, IN_F) and coeffs.shape == (OUT_F, IN_F, 12)
    t = np.linspace(0.0, 1.0, 10, dtype=np.float32)  # same knots as reference

    # Segment index via the same float32 comparisons the reference uses.
    xf = np.ascontiguousarray(x.reshape(-1, IN_F))          # [16384, 512]
    seg = np.zeros(xf.shape, dtype=np.int32)
    for m in range(1, 9):
        seg += (xf >= t[m]).astype(np.int32)

    # Table build (see module docstring): scale-free e4m3; phi1/phi2 = top
    # eigenvectors of the m!=4 covariance, fp8-snapped (phi2 values kept
    # distinct for the device is_equal builds); loadings quantized first
    # (absorbed), residual quantized last, res[4] pinned 0. phi2 covers only
    # the first jc-pair; the second refits with phi1 alone.
    c = coeffs.astype(np.float64)
    F = np.stack(
        [c[:, :, m] + c[:, :, m + 1] + c[:, :, m + 2] for m in range(9)]
    ).reshape(9, -1)                                         # [9, OUT*IN]
    D = F - F[4:5]
    idx = [0, 1, 2, 3, 5, 6, 7, 8]
    C8 = (D[idx] @ D[idx].T) / D.shape[1]
    _, V = np.linalg.eigh(C8)
    grid = _fp8_grid()
    phis = []
    for k in (-1, -2):
        ph = np.zeros(9)
        ph[idx] = V[:, k]
        phis.append(_snap_phi(ph))
    phi1, phi2 = phis
    # ensure phi2 values at m=7,8 are unique (needed for is_equal builds)
    for m in (7, 8):
        others = set(np.delete(phi2, m).tolist())
        if phi2[m] in others:
            gi = int(np.searchsorted(grid, phi2[m]))
            for step in (1, -1, 2, -2, 3, -3):
                cand = grid[(gi + step) % len(grid)]
                if cand not in others and cand != 0.0:
                    phi2[m] = cand
                    break
    assert len(set(phi2[idx].tolist())) == len(idx)

    Phi = np.stack([phi1, phi2], axis=1)                     # [9, 2]
    co2 = np.linalg.lstsq(Phi[idx], D[idx], rcond=None)[0]
    co1only = np.linalg.lstsq(phi1[idx, None], D[idx], rcond=None)[0]
    co1 = co2[0].reshape(OUT_F, IN_F).copy()
    cop2 = co2[1].reshape(OUT_F, IN_F).copy()
    co1[:, 256:] = co1only.reshape(OUT_F, IN_F)[:, 256:]
    cop2[:, 256:] = 0.0

    def q8(v):
        return np.clip(v, -240.0, 240.0).astype(E4NP)

    co1b = q8(co1)
    co2b = q8(cop2)
    res = D.reshape(9, OUT_F, IN_F) \
        - co1b.astype(np.float64)[None] * phi1[:, None, None] \
        - co2b.astype(np.float64)[None] * phi2[:, None, None]
    Rb = q8(res)
    Rb[4] = 0

    # g tables in unit order: phi1 q0/q1, oh m0..m3, m5 (q0/q1 each),
    # m6 q0/q1, phi2 q0, m7 q0/q1, m8 q0/q1.
    unit_tabs = [(co1b, 0), (co1b, 1)]
    for m in (0, 1, 2, 3, 5, 6):
        unit_tabs += [(Rb[m], 0), (Rb[m], 1)]
    unit_tabs.insert(14, (co2b, 0))
    unit_tabs += [(Rb[7], 0), (Rb[7], 1), (Rb[8], 0), (Rb[8], 1)]
    assert len(unit_tabs) == NU
    g_dev = np.empty((128, NU, 2, NPASS, 128), dtype=E4NP)
    packed = {}
    for u, (tab, q) in enumerate(unit_tabs):
        kid = id(tab)
        if kid not in packed:
            packed[kid] = _pack_pair(tab)
        g_dev[:, u] = packed[kid][:, q]
    g_dev = np.ascontiguousarray(g_dev)

    base = F[4].reshape(OUT_F, IN_F).sum(axis=1)             # exact fp32
    sb = np.empty((128, NPASS), dtype=np.float32)
    for ob in range(NPASS):
        sb[:, ob] = base[ob * 128:(ob + 1) * 128]

    # Plane bytes via uint8 LUTs over seg (fast). Slot order: phi1, m0, m1,
    # m2, phi2, m3, m5, m6.
    planes = np.empty((128, NSHIP, JC, seg.shape[0]), dtype=E4NP)
    slot_vals = [phi1, None, None, None, phi2, None, None, None]
    for m, slot in SLOT_OF_OH.items():
        lut = np.zeros(9, E4NP)
        lut[m] = 1.0
        slot_vals[slot] = lut.astype(np.float64)
    for slot, vals in enumerate(slot_vals):
        lut = vals.astype(E4NP).view(np.uint8)
        planes[:, slot] = _plane_dev(lut[seg]).view(E4NP)

    in_maps = []
    for core in range(N_CORES):
        sl = planes[:, :, :, core * TOK:(core + 1) * TOK]
        in_maps.append(
            {
                "pl": np.ascontiguousarray(sl),
                "g": g_dev,
                "sb": sb,
            }
        )

    phi2_cmp = {7: float(phi2[7]), 8: float(phi2[8])}
    nc = _get_program(phi2_cmp)
    res_ = run_bass_kernel_spmd(nc, in_maps, core_ids=list(range(N_CORES)))
    out = np.stack(
        [
            res_.results[core]["out"].reshape(OUT_F, TOK).T.astype(np.float32)
            for core in range(N_CORES)
        ]
    )
    return np.ascontiguousarray(out)


# revision 19
# speedup vs baseline: 1.0078x; 1.0078x over previous
"""Trainium2 Bass kernel for nn_KANLayer (piecewise-constant KAN forward).

Math: reference computes out[t,i] = sum_j f[i,j,m(x_tj)] where m = segment(x)
in 0..8 and f[i,j,m] = c_m + c_{m+1} + c_{m+2} (9-valued selection). The whole
contraction runs in fp8-e4m3 DoubleRow (K=256 per 213ns N=512 matmul):

    out[t,i] = base_i + sum_a co_a[i,j]*phi_a(m_tj)   eigen-planes
             + sum_{m!=4} R[i,j,m] * onehot_m(t,j)    8 planes, 16 units

phi1/phi2 are the top-2 eigenvectors of the (m!=4) covariance of
D = f - f(4), snapped to fp8-exact values (they capture ~83%% of residual
variance vs ~64%% for a lin/quad pair). phi1 covers both j-halves (2 units);
phi2 only the first jc-pair (1 unit, 19 units total) -- the second half's
loadings refit with phi1 alone; host-sim rel err 1.72e-2 vs the 2e-2 gate.
Tables are quantized scale-free (fp8 is floating point, per-row scaling buys
nothing), with the eigen loadings quantized first so their error is absorbed
by the later-quantized one-hot residual R; residual at m=4 is exactly zero
(base anchored at f(4)), so the m=4 plane is dropped. The m7/m8 one-hot
planes are device-built (DVE bf16 is_equal on the phi2 plane -- its snapped
values are kept distinct -- then ACT copy-convert to fp8); everything else
ships as raw e4m3 bytes from host.

Schedule per core: 19 units x 4 out-blocks x 4 token-groups = 304 DR matmuls
(~65us PE). Every (ob, tg) accumulator splits into gen-A (12 early-DMA
units) and gen-B (7 late units: m6 + phi2 + device-built m7/m8). gen-A
partials spill to SBUF f32 with the output bias pre-added (one DVE op), so
the PE has 16 real work streams on 8 PSUM banks and needs no warmup spin
while input DMA ramps. Input pieces complete FIFO per issue queue and the
16 shared rings wake in fixed waves (~2.6/5.5/8.5us), so planes ship as
524KB jc-pair pieces (4KB/partition lines) split q0->sync / q1->scalar in
unit-stream order with each unit-pair's 262KB table chunk interleaved just
ahead; only the tiny bias vector rides the slow gpsimd queue. gen-B runs
accumulator-outer so final evacuations (one DVE tensor_tensor add each)
stagger instead of bursting after the last matmul; out DMA is batched per
(ob, tg-pair) and the last slice evacuates in two pipelined quarters.
Output leaves as [out_block, 128i, tok] bf16, upcast/transposed on host.
Sharding: data-parallel over tokens, 2048 per core; tables replicated.
"""

from contextlib import ExitStack

import numpy as np

import concourse.bass as bass  # noqa: F401
import concourse.tile as tile
from concourse import bacc, mybir
from concourse.bass_utils import run_bass_kernel_spmd

N_CORES = 8
TOK = 2048          # tokens per core
IN_F = 512
OUT_F = 512
JC = IN_F // 128    # 4 j-chunks of 128
NPASS = OUT_F // 128  # 4 out-blocks
NTG = 4             # token groups (N=512 matmuls) per out-block
TGW = TOK // NTG
NU = 19             # DR units: phi1 x2 + phi2 x1 + 16 one-hot (m!=4)
UA = 12             # gen-A units: phi1, oh m0..m3, m5
NSHIP = 8           # shipped planes
FP8 = mybir.dt.float8e4
BF16 = mybir.dt.bfloat16
F32 = mybir.dt.float32
E4NP = mybir.dt.np(FP8)  # ml_dtypes.float8_e4m3 (TRN: bias 7, max 240)

# plane slots (pl tensor): 0=phi1, 1=oh m0, 2=oh m1, 3=oh m2, 4=phi2,
# 5=oh m3, 6=oh m5, 7=oh m6, 8=oh m7 (device), 9=oh m8 (device).
# 2-plane DMA pieces: [0:2], [2:4], [4:6], [6:8].
SLOT_OF_OH = {0: 1, 1: 2, 2: 3, 3: 5, 5: 6, 6: 7}  # shipped one-hots

# unit -> (plane slot, jc-pair q), PE stream order. gen-A: phi1, m0..m3, m5
# (12). gen-B: m6, phi2 (q0 only), m7, m8 (7).
_UNITS = [(pk, q) for pk in (0, 1, 2, 3, 5, 6) for q in range(2)] \
    + [(7, 0), (7, 1), (4, 0), (8, 0), (8, 1), (9, 0), (9, 1)]
assert len(_UNITS) == NU

_PROGRAM_CACHE = {}


def _build_program():
    nc = bacc.Bacc("TRN2", target_bir_lowering=False, debug=False)

    pl_d = nc.dram_tensor("pl", [128, NSHIP, JC, TOK], FP8,
                          kind="ExternalInput").ap()
    g_d = nc.dram_tensor("g", [128, NU, 2, NPASS, 128], FP8,
                         kind="ExternalInput").ap()
    sb_d = nc.dram_tensor("sb", [128, NPASS], F32, kind="ExternalInput").ap()
    out_d = nc.dram_tensor("out", [NPASS, 128, TOK], BF16,
                           kind="ExternalOutput").ap()

    with tile.TileContext(nc) as tc, ExitStack() as ctx:
        tmp_pool = ctx.enter_context(tc.tile_pool(name="tmp", bufs=2))
        pl_pool = ctx.enter_context(tc.tile_pool(name="pl", bufs=1))
        g_pool = ctx.enter_context(tc.tile_pool(name="g", bufs=1))
        sb_pool = ctx.enter_context(tc.tile_pool(name="sb", bufs=1))
        part_pool = ctx.enter_context(tc.tile_pool(name="part", bufs=1))
        out_pool = ctx.enter_context(tc.tile_pool(name="out", bufs=4))
        psum_pool = ctx.enter_context(tc.tile_pool(name="psum", bufs=8,
                                                   space="PSUM"))

        # --- input DMAs: few big pieces (descriptor-rate-bound frontend).
        # Planes as four 2-plane pieces in unit-stream order on the two
        # hwdge queues; all tables as one piece + sb on the gpsimd queue.
        pl_t = pl_pool.tile([128, NSHIP + 2, JC, TOK], FP8, name="pl")
        g_t = g_pool.tile([128, NU, 2, NPASS, 128], FP8, name="g")
        sb_t = sb_pool.tile([128, NPASS], F32, name="sb")

        # Queue model (measured): pieces complete FIFO per queue; sync rings
        # wake ~2.7/5.7us, scalar ~8.9us, gpsimd late+slow (only sb goes
        # there). Planes ship as jc-pair halves (524KB, 4KB lines) split
        # q0->sync / q1->scalar in unit-stream order; each unit-pair's
        # tables (262KB chunk) ride the opposite queue just ahead.
        def pq(eng, pk, q):
            eng.dma_start(pl_t[:, pk, 2 * q:2 * q + 2],
                          pl_d[:, pk, 2 * q:2 * q + 2])

        def gq(eng, u0, u1):
            eng.dma_start(g_t[:, u0:u1], g_d[:, u0:u1])

        pq(nc.sync, 0, 0)          # phi1 q0
        pq(nc.scalar, 0, 1)        # phi1 q1
        gq(nc.sync, 0, 1)          # unit-0 table (minimal first-MM gate)
        gq(nc.scalar, 1, 2)        # unit-1 table
        gq(nc.scalar, 2, 4)        # m0 tables
        pq(nc.sync, 1, 0)          # m0 q0
        pq(nc.scalar, 1, 1)        # m0 q1
        gq(nc.sync, 4, 6)          # m1 tables
        gq(nc.scalar, 6, 8)        # m2 tables
        pq(nc.sync, 2, 0)          # m1 q0
        pq(nc.scalar, 2, 1)        # m1 q1
        pq(nc.sync, 3, 0)          # m2 q0
        pq(nc.scalar, 3, 1)        # m2 q1
        gq(nc.sync, 8, 10)         # m3 tables
        nc.gpsimd.dma_start(sb_t[:], sb_d[:])
        pq(nc.scalar, 4, 0)        # phi2 q0 (ACT builds + unit 14)
        pq(nc.sync, 5, 0)          # m3 q0
        pq(nc.scalar, 5, 1)        # m3 q1
        gq(nc.sync, 10, 12)        # m5 tables
        pq(nc.scalar, 4, 1)        # phi2 q1 (ACT builds)
        pq(nc.sync, 6, 0)          # m5 q0
        pq(nc.scalar, 6, 1)        # m5 q1
        gq(nc.sync, 12, 14)        # m6 tables
        gq(nc.scalar, 14, 15)      # phi2 table
        pq(nc.sync, 7, 0)          # m6 q0
        pq(nc.scalar, 7, 1)        # m6 q1
        gq(nc.sync, 15, 19)        # m7/m8 tables

        def build_planes():
            # Device-built planes: one-hot m7/m8 via DVE bf16 is_equal on
            # the phi2 plane (values kept distinct host-side) + ACT
            # copy-convert to fp8, per jc-pair chunk. Emitted AFTER the
            # gen-A h=0 partial evacuations so the ~7us of is_equal work
            # can never delay the bank-freeing evacs on the Vector queue
            # (a race observed as a 2.3us PE gap on some runs).
            for slot, mval in ((8, 7), (9, 8)):
                for q in range(2):
                    tmp = tmp_pool.tile([128, 2, TOK], BF16, name="ohb")
                    nc.vector.tensor_scalar(
                        tmp[:], pl_t[:, 4, 2 * q:2 * q + 2],
                        _PROGRAM_CACHE["phi2_cmp"][mval], None,
                        mybir.AluOpType.is_equal,
                    )
                    nc.scalar.activation(
                        pl_t[:, slot, 2 * q:2 * q + 2],
                        tmp[:],
                        mybir.ActivationFunctionType.Copy,
                    )

        def mm(ps, ob, u, tg, start, stop):
            pk, q = _UNITS[u]
            nc.tensor.matmul(
                ps,
                g_t[:, u, :, ob, :],
                pl_t[:, pk, 2 * q:2 * q + 2, tg * TGW:(tg + 1) * TGW],
                start=start,
                stop=stop,
                perf_mode=mybir.MatmulPerfMode.DoubleRow,
            )

        # part_t[:, h, ob, tgi*TGW:...] = gen-A partial + bias for (ob, tg)
        part_t = part_pool.tile([128, 2, NPASS, TGW * 2], F32, name="part")

        # gen-A: unit-outer (DMA arrival order), tg halves h=0 then h=1.
        pss = {}
        for h in (0, 1):
            for u in range(UA):
                for ob in range(NPASS):
                    for tgi in (0, 1):
                        if u == 0:
                            pss[ob, tgi] = psum_pool.tile(
                                [128, TGW], F32, name="ps")
                        mm(pss[ob, tgi][:], ob, u, 2 * h + tgi,
                           start=(u == 0), stop=(u == UA - 1))
            for ob in range(NPASS):
                for tgi in (0, 1):
                    nc.vector.tensor_scalar(
                        part_t[:, h, ob, tgi * TGW:(tgi + 1) * TGW],
                        pss[ob, tgi][:], sb_t[:, ob:ob + 1], None,
                        mybir.AluOpType.add,
                    )
            if h == 0:
                build_planes()

        # gen-B: accumulator-outer so evacs stagger (one DVE add each); out
        # DMA batched per (h, ob) except the last pair, whose second half
        # evacuates in two pipelined quarters for the shortest exposed tail.
        for h in (0, 1):
            for ob in range(NPASS):
                otb = out_pool.tile([128, 2 * TGW], BF16, name="otb")
                last = (h == 1 and ob == NPASS - 1)
                eng = nc.sync if ob % 2 == 0 else nc.scalar
                for tgi in (0, 1):
                    tg = 2 * h + tgi
                    ps = psum_pool.tile([128, TGW], F32, name="ps")
                    for ui, u in enumerate(range(UA, NU)):
                        mm(ps[:], ob, u, tg,
                           start=(ui == 0), stop=(u == NU - 1))
                    nhalf = 2 if (last and tgi == 1) else 1
                    hw = TGW // nhalf
                    for hh in range(nhalf):
                        sl = slice(tgi * TGW + hh * hw,
                                   tgi * TGW + (hh + 1) * hw)
                        nc.vector.tensor_tensor(
                            otb[:, sl], ps[:, hh * hw:(hh + 1) * hw],
                            part_t[:, h, ob, sl],
                            mybir.AluOpType.add,
                        )
                        if last:
                            # alternate queues so the two final pieces'
                            # issue latencies overlap
                            leng = nc.scalar if hh == 0 else nc.sync
                            leng.dma_start(
                                out_d[ob][:, 2 * h * TGW:][:, sl],
                                otb[:, sl])
                if not last:
                    eng.dma_start(
                        out_d[ob][:, 2 * h * TGW:2 * (h + 1) * TGW], otb[:])

    nc.compile()
    return nc


def _get_program(phi2_cmp=None):
    # phi2 compare constants are baked into the program; rebuild if they
    # change (same coeffs -> same program).
    if phi2_cmp is None:
        return _PROGRAM_CACHE["nc"]
    key = ("nc", tuple(sorted(phi2_cmp.items())))
    if _PROGRAM_CACHE.get("key") != key:
        _PROGRAM_CACHE["phi2_cmp"] = phi2_cmp
        _PROGRAM_CACHE["nc"] = _build_program()
        _PROGRAM_CACHE["key"] = key
    return _PROGRAM_CACHE["nc"]


def _plane_dev(arr):
    """[T_all, IN] -> [128, JC, T_all] device layout (j = jc*128 + p)."""
    return np.ascontiguousarray(arr.T.reshape(JC, 128, -1).transpose(1, 0, 2))


def _pack_pair(tab_b):
    """e4m3 [OUT, IN] -> [128p, 2q, 2e, NPASS, 128col] stationary layout."""
    t = tab_b.reshape(NPASS, 128, JC, 128).transpose(3, 2, 0, 1)
    return np.ascontiguousarray(t.reshape(128, 2, 2, NPASS, 128))


def _fp8_grid():
    b = np.arange(256, dtype=np.uint8).view(E4NP).astype(np.float64)
    return np.unique(b[np.isfinite(b)])


def _snap_phi(phi):
    """Snap phi (phi[4]=0 preserved) to fp8-exact values, scaled to ~12."""
    ph = (phi * (12.0 / np.abs(phi).max())).astype(E4NP).astype(np.float64)
    ph[4] = 0.0
    return ph


def kernel(x: np.ndarray, coeffs: np.ndarray) -> np.ndarray:
    assert x.shape == (8, 2048, IN_F) and coeffs.shape == (OUT_F, IN_F, 12)
    t = np.linspace(0.0, 1.0, 10, dtype=np.float32)  # same knots as reference

    # Segment index via the same float32 comparisons the reference uses.
    xf = np.ascontiguousarray(x.reshape(-1, IN_F))          # [16384, 512]
    seg = np.zeros(xf.shape, dtype=np.int32)
    for m in range(1, 9):
        seg += (xf >= t[m]).astype(np.int32)

    # Table build (see module docstring): scale-free e4m3; phi1/phi2 = top
    # eigenvectors of the m!=4 covariance, fp8-snapped (phi2 values kept
    # distinct for the device is_equal builds); loadings quantized first
    # (absorbed), residual quantized last, res[4] pinned 0. phi2 covers only
    # the first jc-pair; the second refits with phi1 alone.
    c = coeffs.astype(np.float64)
    F = np.stack(
        [c[:, :, m] + c[:, :, m + 1] + c[:, :, m + 2] for m in range(9)]
    ).reshape(9, -1)                                         # [9, OUT*IN]
    D = F - F[4:5]
    idx = [0, 1, 2, 3, 5, 6, 7, 8]
    C8 = (D[idx] @ D[idx].T) / D.shape[1]
    _, V = np.linalg.eigh(C8)
    grid = _fp8_grid()
    phis = []
    for k in (-1, -2):
        ph = np.zeros(9)
        ph[idx] = V[:, k]
        phis.append(_snap_phi(ph))
    phi1, phi2 = phis
    # ensure phi2 values at m=7,8 are unique (needed for is_equal builds)
    for m in (7, 8):
        others = set(np.delete(phi2, m).tolist())
        if phi2[m] in others:
            gi = int(np.searchsorted(grid, phi2[m]))
            for step in (1, -1, 2, -2, 3, -3):
                cand = grid[(gi + step) % len(grid)]
                if cand not in others and cand != 0.0:
                    phi2[m] = cand
                    break
    assert len(set(phi2[idx].tolist())) == len(idx)

    Phi = np.stack([phi1, phi2], axis=1)                     # [9, 2]
    co2 = np.linalg.lstsq(Phi[idx], D[idx], rcond=None)[0]
    co1only = np.linalg.lstsq(phi1[idx, None], D[idx], rcond=None)[0]
    co1 = co2[0].reshape(OUT_F, IN_F).copy()
    cop2 = co2[1].reshape(OUT_F, IN_F).copy()
    co1[:, 256:] = co1only.reshape(OUT_F, IN_F)[:, 256:]
    cop2[:, 256:] = 0.0

    def q8(v):
        return np.clip(v, -240.0, 240.0).astype(E4NP)

    co1b = q8(co1)
    co2b = q8(cop2)
    res = D.reshape(9, OUT_F, IN_F) \
        - co1b.astype(np.float64)[None] * phi1[:, None, None] \
        - co2b.astype(np.float64)[None] * phi2[:, None, None]
    Rb = q8(res)
    Rb[4] = 0

    # g tables in unit order: phi1 q0/q1, oh m0..m3, m5 (q0/q1 each),
    # m6 q0/q1, phi2 q0, m7 q0/q1, m8 q0/q1.
    unit_tabs = [(co1b, 0), (co1b, 1)]
    for m in (0, 1, 2, 3, 5, 6):
        unit_tabs += [(Rb[m], 0), (Rb[m], 1)]
    unit_tabs.insert(14, (co2b, 0))
    unit_tabs += [(Rb[7], 0), (Rb[7], 1), (Rb[8], 0), (Rb[8], 1)]
    assert len(unit_tabs) == NU
    g_dev = np.empty((128, NU, 2, NPASS, 128), dtype=E4NP)
    packed = {}
    for u, (tab, q) in enumerate(unit_tabs):
        kid = id(tab)
        if kid not in packed:
            packed[kid] = _pack_pair(tab)
        g_dev[:, u] = packed[kid][:, q]
    g_dev = np.ascontiguousarray(g_dev)

    base = F[4].reshape(OUT_F, IN_F).sum(axis=1)             # exact fp32
    sb = np.empty((128, NPASS), dtype=np.float32)
    for ob in range(NPASS):
        sb[:, ob] = base[ob * 128:(ob + 1) * 128]

    # Plane bytes via uint8 LUTs over seg (fast). Slot order: phi1, m0, m1,
    # m2, phi2, m3, m5, m6.
    planes = np.empty((128, NSHIP, JC, seg.shape[0]), dtype=E4NP)
    slot_vals = [phi1, None, None, None, phi2, None, None, None]
    for m, slot in SLOT_OF_OH.items():
        lut = np.zeros(9, E4NP)
        lut[m] = 1.0
        slot_vals[slot] = lut.astype(np.float64)
    for slot, vals in enumerate(slot_vals):
        lut = vals.astype(E4NP).view(np.uint8)
        planes[:, slot] = _plane_dev(lut[seg]).view(E4NP)

    in_maps = []
    for core in range(N_CORES):
        sl = planes[:, :, :, core * TOK:(core + 1) * TOK]
        in_maps.append(
            {
                "pl": np.ascontiguousarray(sl),
                "g": g_dev,
                "sb": sb,
            }
        )

    phi2_cmp = {7: float(phi2[7]), 8: float(phi2[8])}
    nc = _get_program(phi2_cmp)
    res_ = run_bass_kernel_spmd(nc, in_maps, core_ids=list(range(N_CORES)))
    out = np.stack(
        [
            res_.results[core]["out"].reshape(OUT_F, TOK).T.astype(np.float32)
            for core in range(N_CORES)
        ]
    )
    return np.ascontiguousarray(out)


# revision 22
# speedup vs baseline: 1.0573x; 1.0491x over previous
"""Trainium2 Bass kernel for nn_KANLayer (piecewise-constant KAN forward).

Math: reference computes out[t,i] = sum_j f[i,j,m(x_tj)] where m = segment(x)
in 0..8 and f[i,j,m] = c_m + c_{m+1} + c_{m+2} (9-valued selection). The whole
contraction runs in fp8-e4m3 DoubleRow (K=256 per 213ns N=512 matmul):

    out[t,i] = base_i + sum_a co_a[i,j]*phi_a(m_tj)   eigen-planes
             + sum_{m!=4} R[i,j,m] * onehot_m(t,j)    8 planes, 16 units

phi1/phi2 are the top-2 eigenvectors of the (m!=4) covariance of
D = f - f(4), snapped to fp8-exact values (they capture ~83%% of residual
variance vs ~64%% for a lin/quad pair). phi1 covers both j-halves (2 units);
phi2 only the first jc-pair (1 unit, 19 units total) -- the second half's
loadings refit with phi1 alone; host-sim rel err 1.72e-2 vs the 2e-2 gate.
Tables are quantized scale-free (fp8 is floating point, per-row scaling buys
nothing), with the eigen loadings quantized first so their error is absorbed
by the later-quantized one-hot residual R; residual at m=4 is exactly zero
(base anchored at f(4)), so the m=4 plane is dropped. The m7/m8 one-hot
planes are device-built (DVE bf16 is_equal on the phi2 plane -- its snapped
values are kept distinct -- then ACT copy-convert to fp8); everything else
ships as raw e4m3 bytes from host.

Schedule per core: 19 units x 4 out-blocks x 4 token-groups = 304 DR matmuls
(~65us PE). Every (ob, tg) accumulator splits into gen-A (12 early-DMA
units) and gen-B (7 late units: m6 + phi2 + device-built m7/m8). gen-A
partials spill to SBUF f32 with the output bias pre-added (one DVE op), so
the PE has 16 real work streams on 8 PSUM banks and needs no warmup spin
while input DMA ramps. Input pieces complete FIFO per issue queue and the
16 shared rings wake in fixed waves (~2.6/5.5/8.5us), so planes ship as
524KB jc-pair pieces (4KB/partition lines) split q0->sync / q1->scalar in
unit-stream order with each unit-pair's 262KB table chunk interleaved just
ahead; only the tiny bias vector rides the slow gpsimd queue. gen-B runs
accumulator-outer so final evacuations (one DVE tensor_tensor add each)
stagger instead of bursting after the last matmul; out DMA is batched per
(ob, tg-pair) and the last slice evacuates in two pipelined quarters.
Output leaves as [out_block, 128i, tok] bf16, upcast/transposed on host.
Sharding: data-parallel over tokens, 2048 per core; tables replicated.
"""

from contextlib import ExitStack

import numpy as np

import concourse.bass as bass  # noqa: F401
import concourse.tile as tile
from concourse import bacc, mybir
from concourse.bass_utils import run_bass_kernel_spmd

N_CORES = 8
TOK = 2048          # tokens per core
IN_F = 512
OUT_F = 512
JC = IN_F // 128    # 4 j-chunks of 128
NPASS = OUT_F // 128  # 4 out-blocks
NTG = 4             # token groups (N=512 matmuls) per out-block
TGW = TOK // NTG
NU = 19             # DR units: phi1 x2 + phi2 x1 + 16 one-hot (m!=4)
UA = 12             # gen-A units: phi1, oh m0..m3, m5
NSHIP = 8           # shipped planes
FP8 = mybir.dt.float8e4
BF16 = mybir.dt.bfloat16
F32 = mybir.dt.float32
E4NP = mybir.dt.np(FP8)  # ml_dtypes.float8_e4m3 (TRN: bias 7, max 240)

# plane slots (pl tensor): 0=phi1, 1=oh m0, 2=oh m1, 3=oh m2, 4=phi2,
# 5=oh m3, 6=oh m5, 7=oh m6, 8=oh m7 (device), 9=oh m8 (device).
# 2-plane DMA pieces: [0:2], [2:4], [4:6], [6:8].
SLOT_OF_OH = {0: 1, 1: 2, 2: 3, 3: 5, 5: 6, 6: 7}  # shipped one-hots

# unit -> (plane slot, jc-pair q), PE stream order. gen-A: phi1, m0..m3, m5
# (12). gen-B: m6, phi2 (q0 only), m7, m8 (7).
_UNITS = [(pk, q) for pk in (0, 1, 2, 3, 5, 6) for q in range(2)] \
    + [(7, 0), (7, 1), (4, 0), (8, 0), (8, 1), (9, 0), (9, 1)]
assert len(_UNITS) == NU

_PROGRAM_CACHE = {}


def _build_program():
    nc = bacc.Bacc("TRN2", target_bir_lowering=False, debug=False)

    pl_d = nc.dram_tensor("pl", [128, NSHIP, JC, TOK], FP8,
                          kind="ExternalInput").ap()
    g_d = nc.dram_tensor("g", [128, NU, 2, NPASS, 128], FP8,
                         kind="ExternalInput").ap()
    sb_d = nc.dram_tensor("sb", [128, NPASS], F32, kind="ExternalInput").ap()
    out_d = nc.dram_tensor("out", [NPASS, 128, TOK], BF16,
                           kind="ExternalOutput").ap()

    with tile.TileContext(nc) as tc, ExitStack() as ctx:
        tmp_pool = ctx.enter_context(tc.tile_pool(name="tmp", bufs=2))
        pl_pool = ctx.enter_context(tc.tile_pool(name="pl", bufs=1))
        g_pool = ctx.enter_context(tc.tile_pool(name="g", bufs=1))
        sb_pool = ctx.enter_context(tc.tile_pool(name="sb", bufs=1))
        part_pool = ctx.enter_context(tc.tile_pool(name="part", bufs=1))
        out_pool = ctx.enter_context(tc.tile_pool(name="out", bufs=4))
        psum_pool = ctx.enter_context(tc.tile_pool(name="psum", bufs=8,
                                                   space="PSUM"))

        # --- input DMAs: few big pieces (descriptor-rate-bound frontend).
        # Planes as four 2-plane pieces in unit-stream order on the two
        # hwdge queues; all tables as one piece + sb on the gpsimd queue.
        pl_t = pl_pool.tile([128, NSHIP + 2, JC, TOK], FP8, name="pl")
        g_t = g_pool.tile([128, NU, 2, NPASS, 128], FP8, name="g")
        sb_t = sb_pool.tile([128, NPASS], F32, name="sb")

        # Queue model (measured): pieces complete FIFO per queue; sync rings
        # wake ~2.7/5.7us, scalar ~8.9us, gpsimd late+slow (only sb goes
        # there). Planes ship as jc-pair halves (524KB, 4KB lines) split
        # q0->sync / q1->scalar in unit-stream order; each unit-pair's
        # tables (262KB chunk) ride the opposite queue just ahead.
        def pq(eng, pk, q):
            eng.dma_start(pl_t[:, pk, 2 * q:2 * q + 2],
                          pl_d[:, pk, 2 * q:2 * q + 2])

        def gq(eng, u0, u1):
            eng.dma_start(g_t[:, u0:u1], g_d[:, u0:u1])

        pq(nc.sync, 0, 0)          # phi1 q0
        pq(nc.scalar, 0, 1)        # phi1 q1
        gq(nc.sync, 0, 1)          # unit-0 table (minimal first-MM gate)
        gq(nc.scalar, 1, 2)        # unit-1 table
        gq(nc.scalar, 2, 4)        # m0 tables
        pq(nc.sync, 1, 0)          # m0 q0
        pq(nc.scalar, 1, 1)        # m0 q1
        gq(nc.sync, 4, 6)          # m1 tables
        gq(nc.scalar, 6, 8)        # m2 tables
        pq(nc.sync, 2, 0)          # m1 q0
        pq(nc.scalar, 2, 1)        # m1 q1
        pq(nc.sync, 3, 0)          # m2 q0
        pq(nc.scalar, 3, 1)        # m2 q1
        gq(nc.sync, 8, 10)         # m3 tables
        nc.gpsimd.dma_start(sb_t[:], sb_d[:])
        pq(nc.scalar, 4, 0)        # phi2 q0 (ACT builds + unit 14)
        pq(nc.sync, 5, 0)          # m3 q0
        pq(nc.scalar, 5, 1)        # m3 q1
        gq(nc.sync, 10, 12)        # m5 tables
        pq(nc.scalar, 4, 1)        # phi2 q1 (ACT builds)
        pq(nc.sync, 6, 0)          # m5 q0
        pq(nc.scalar, 6, 1)        # m5 q1
        gq(nc.sync, 12, 14)        # m6 tables
        gq(nc.scalar, 14, 15)      # phi2 table
        pq(nc.sync, 7, 0)          # m6 q0
        pq(nc.scalar, 7, 1)        # m6 q1
        gq(nc.sync, 15, 19)        # m7/m8 tables

        def build_planes():
            # Device-built planes: one-hot m7/m8 via DVE bf16 is_equal on
            # the phi2 plane (values kept distinct host-side) + ACT
            # copy-convert to fp8, per jc-pair chunk. Emitted AFTER the
            # gen-A h=0 partial evacuations so the ~7us of is_equal work
            # can never delay the bank-freeing evacs on the Vector queue
            # (a race observed as a 2.3us PE gap on some runs).
            for slot, mval in ((8, 7), (9, 8)):
                for q in range(2):
                    tmp = tmp_pool.tile([128, 2, TOK], BF16, name="ohb")
                    nc.vector.tensor_scalar(
                        tmp[:], pl_t[:, 4, 2 * q:2 * q + 2],
                        _PROGRAM_CACHE["phi2_cmp"][mval], None,
                        mybir.AluOpType.is_equal,
                    )
                    nc.scalar.activation(
                        pl_t[:, slot, 2 * q:2 * q + 2],
                        tmp[:],
                        mybir.ActivationFunctionType.Copy,
                    )

        def mm(ps, ob, u, tg, start, stop):
            pk, q = _UNITS[u]
            nc.tensor.matmul(
                ps,
                g_t[:, u, :, ob, :],
                pl_t[:, pk, 2 * q:2 * q + 2, tg * TGW:(tg + 1) * TGW],
                start=start,
                stop=stop,
                perf_mode=mybir.MatmulPerfMode.DoubleRow,
            )

        # part_t[:, h, ob, tgi*TGW:...] = gen-A partial + bias for (ob, tg)
        part_t = part_pool.tile([128, 2, NPASS, TGW * 2], F32, name="part")

        # gen-A: unit-outer (DMA arrival order), tg halves h=0 then h=1.
        pss = {}
        for h in (0, 1):
            for u in range(UA):
                for ob in range(NPASS):
                    for tgi in (0, 1):
                        if u == 0:
                            pss[ob, tgi] = psum_pool.tile(
                                [128, TGW], F32, name="ps")
                        mm(pss[ob, tgi][:], ob, u, 2 * h + tgi,
                           start=(u == 0), stop=(u == UA - 1))
            for ob in range(NPASS):
                for tgi in (0, 1):
                    nc.vector.tensor_scalar(
                        part_t[:, h, ob, tgi * TGW:(tgi + 1) * TGW],
                        pss[ob, tgi][:], sb_t[:, ob:ob + 1], None,
                        mybir.AluOpType.add,
                    )
            if h == 0:
                build_planes()

        # gen-B: accumulator-outer so evacs stagger (one DVE add each); out
        # DMA batched per (h, ob) except the last pair, whose second half
        # evacuates in two pipelined quarters for the shortest exposed tail.
        for h in (0, 1):
            for ob in range(NPASS):
                otb = out_pool.tile([128, 2 * TGW], BF16, name="otb")
                last = (h == 1 and ob == NPASS - 1)
                eng = nc.sync if ob % 2 == 0 else nc.scalar
                for tgi in (0, 1):
                    tg = 2 * h + tgi
                    ps = psum_pool.tile([128, TGW], F32, name="ps")
                    for ui, u in enumerate(range(UA, NU)):
                        mm(ps[:], ob, u, tg,
                           start=(ui == 0), stop=(u == NU - 1))
                    nhalf = 2 if (last and tgi == 1) else 1
                    hw = TGW // nhalf
                    for hh in range(nhalf):
                        sl = slice(tgi * TGW + hh * hw,
                                   tgi * TGW + (hh + 1) * hw)
                        nc.vector.tensor_tensor(
                            otb[:, sl], ps[:, hh * hw:(hh + 1) * hw],
                            part_t[:, h, ob, sl],
                            mybir.AluOpType.add,
                        )
                        if last:
                            # alternate queues so the two final pieces'
                            # issue latencies overlap
                            leng = nc.scalar if hh == 0 else nc.sync
                            leng.dma_start(
                                out_d[ob][:, 2 * h * TGW:][:, sl],
                                otb[:, sl])
                if not last:
                    eng.dma_start(
                        out_d[ob][:, 2 * h * TGW:2 * (h + 1) * TGW], otb[:])

    nc.compile()
    return nc


def _get_program(phi2_cmp=None):
    # phi2 compare constants are baked into the program; rebuild if they
    # change (same coeffs -> same program).
    if phi2_cmp is None:
        return _PROGRAM_CACHE["nc"]
    key = ("nc", tuple(sorted(phi2_cmp.items())))
    if _PROGRAM_CACHE.get("key") != key:
        _PROGRAM_CACHE["phi2_cmp"] = phi2_cmp
        _PROGRAM_CACHE["nc"] = _build_program()
        _PROGRAM_CACHE["key"] = key
    return _PROGRAM_CACHE["nc"]


def _plane_dev(arr):
    """[T_all, IN] -> [128, JC, T_all] device layout (j = jc*128 + p)."""
    return np.ascontiguousarray(arr.T.reshape(JC, 128, -1).transpose(1, 0, 2))


def _pack_pair(tab_b):
    """e4m3 [OUT, IN] -> [128p, 2q, 2e, NPASS, 128col] stationary layout."""
    t = tab_b.reshape(NPASS, 128, JC, 128).transpose(3, 2, 0, 1)
    return np.ascontiguousarray(t.reshape(128, 2, 2, NPASS, 128))


def _fp8_grid():
    b = np.arange(256, dtype=np.uint8).view(E4NP).astype(np.float64)
    return np.unique(b[np.isfinite(b)])


def _snap_phi(phi):
    """Snap phi (phi[4]=0 preserved) to fp8-exact values, scaled to ~12."""
    ph = (phi * (12.0 / np.abs(phi).max())).astype(E4NP).astype(np.float64)
    ph[4] = 0.0
    return ph


def kernel(x: np.ndarray, coeffs: np.ndarray) -> np.ndarray:
    assert x.shape == (8, 2048, IN_F) and coeffs.shape == (OUT_F, IN_F, 12)
    t = np.linspace(0.0, 1.0, 10, dtype=np.float32)  # same knots as reference

    # Segment index via the same float32 comparisons the reference uses.
    xf = np.ascontiguousarray(x.reshape(-1, IN_F))          # [16384, 512]
    seg = np.zeros(xf.shape, dtype=np.int32)
    for m in range(1, 9):
        seg += (xf >= t[m]).astype(np.int32)

    # Table build (see module docstring): scale-free e4m3; phi1/phi2 = top
    # eigenvectors of the m!=4 covariance, fp8-snapped (phi2 values kept
    # distinct for the device is_equal builds); loadings quantized first
    # (absorbed), residual quantized last, res[4] pinned 0. phi2 covers only
    # the first jc-pair; the second refits with phi1 alone.
    c = coeffs.astype(np.float64)
    F = np.stack(
        [c[:, :, m] + c[:, :, m + 1] + c[:, :, m + 2] for m in range(9)]
    ).reshape(9, -1)                                         # [9, OUT*IN]
    D = F - F[4:5]
    idx = [0, 1, 2, 3, 5, 6, 7, 8]
    C8 = (D[idx] @ D[idx].T) / D.shape[1]
    _, V = np.linalg.eigh(C8)
    grid = _fp8_grid()
    phis = []
    for k in (-1, -2):
        ph = np.zeros(9)
        ph[idx] = V[:, k]
        phis.append(_snap_phi(ph))
    phi1, phi2 = phis
    # ensure phi2 values at m=7,8 are unique (needed for is_equal builds)
    for m in (7, 8):
        others = set(np.delete(phi2, m).tolist())
        if phi2[m] in others:
            gi = int(np.searchsorted(grid, phi2[m]))
            for step in (1, -1, 2, -2, 3, -3):
                cand = grid[(gi + step) % len(grid)]
                if cand not in others and cand != 0.0:
                    phi2[m] = cand
                    break
    assert len(set(phi2[idx].tolist())) == len(idx)

    Phi = np.stack([phi1, phi2], axis=1)                     # [9, 2]
    co2 = np.linalg.lstsq(Phi[idx], D[idx], rcond=None)[0]
    co1only = np.linalg.lstsq(phi1[idx, None], D[idx], rcond=None)[0]
    co1 = co2[0].reshape(OUT_F, IN_F).copy()
    cop2 = co2[1].reshape(OUT_F, IN_F).copy()
    co1[:, 256:] = co1only.reshape(OUT_F, IN_F)[:, 256:]
    cop2[:, 256:] = 0.0

    def q8(v):
        return np.clip(v, -240.0, 240.0).astype(E4NP)

    co1b = q8(co1)
    co2b = q8(cop2)
    res = D.reshape(9, OUT_F, IN_F) \
        - co1b.astype(np.float64)[None] * phi1[:, None, None] \
        - co2b.astype(np.float64)[None] * phi2[:, None, None]
    Rb = q8(res)
    Rb[4] = 0

    # g tables in unit order: phi1 q0/q1, oh m0..m3, m5 (q0/q1 each),
    # m6 q0/q1, phi2 q0, m7 q0/q1, m8 q0/q1.
    unit_tabs = [(co1b, 0), (co1b, 1)]
    for m in (0, 1, 2, 3, 5, 6):
        unit_tabs += [(Rb[m], 0), (Rb[m], 1)]
    unit_tabs.insert(14, (co2b, 0))
    unit_tabs += [(Rb[7], 0), (Rb[7], 1), (Rb[8], 0), (Rb[8], 1)]
    assert len(unit_tabs) == NU
    g_dev = np.empty((128, NU, 2, NPASS, 128), dtype=E4NP)
    packed = {}
    for u, (tab, q) in enumerate(unit_tabs):
        kid = id(tab)
        if kid not in packed:
            packed[kid] = _pack_pair(tab)
        g_dev[:, u] = packed[kid][:, q]
    g_dev = np.ascontiguousarray(g_dev)

    base = F[4].reshape(OUT_F, IN_F).sum(axis=1)             # exact fp32
    sb = np.empty((128, NPASS), dtype=np.float32)
    for ob in range(NPASS):
        sb[:, ob] = base[ob * 128:(ob + 1) * 128]

    # Plane bytes via uint8 LUTs over seg (fast). Slot order: phi1, m0, m1,
    # m2, phi2, m3, m5, m6.
    planes = np.empty((128, NSHIP, JC, seg.shape[0]), dtype=E4NP)
    slot_vals = [phi1, None, None, None, phi2, None, None, None]
    for m, slot in SLOT_OF_OH.items():
        lut = np.zeros(9, E4NP)
        lut[m] = 1.0
        slot_vals[slot] = lut.astype(np.float64)
    for slot, vals in enumerate(slot_vals):
        lut = vals.astype(E4NP).view(np.uint8)
        planes[:, slot] = _plane_dev(lut[seg]).view(E4NP)

    in_maps = []
    for core in range(N_CORES):
        sl = planes[:, :, :, core * TOK:(core + 1) * TOK]
        in_maps.append(
            {
                "pl": np.ascontiguousarray(sl),
                "g": g_dev,
                "sb": sb,
            }
        )

    phi2_cmp = {7: float(phi2[7]), 8: float(phi2[8])}
    nc = _get_program(phi2_cmp)
    res_ = run_bass_kernel_spmd(nc, in_maps, core_ids=list(range(N_CORES)))
    out = np.stack(
        [
            res_.results[core]["out"].reshape(OUT_F, TOK).T.astype(np.float32)
            for core in range(N_CORES)
        ]
    )
    return np.ascontiguousarray(out)
